# revision 22
# baseline (speedup 1.0000x reference)
"""ColBERT MaxSim contrastive loss on 8 Trainium2 NeuronCores (v3c).

Sharding: doc-parallel (each core scores ALL 64*32 query tokens against its
8-doc shard = 8192 doc tokens). Per core, per M-tile (128 query-token rows):
  - PE: per doc, 2 matmuls [128x128]@[128x512] -> PSUM [128,2,512] (2 banks)
  - "C"-assigned (m, doc) pairs: VectorE tensor_reduce(max) directly on the
    PSUM tile -> dmax[:, c] (bf16).  No ScalarE involvement.
  - "B"-assigned pairs: ScalarE single ACTIVATE exp(BETA*x + ABIAS) with
    accum_out (running sum over the 1024 doc tokens) -> lse_all[:, col].
    Sharp log-sum-exp (BETA=100) stands in for the exact max; ln, token-sum
    and bias correction happen on the HOST (lse_all is [128, NBP] fp32).
    Loss rel err vs the fp32 reference ~4e-4 (gate is 2e-2).
  - dmax is zeroed per M-tile (GPSIMD, otherwise idle) so B columns
    contribute nothing to the n-sum matmul; the host adds the B scores from
    lse_all.  This makes the B/C split free per (m, doc), letting us match
    the 1416ns/doc ScalarE pitch against the 1300ns/doc VectorE pitch
    (60 B / 68 C), instead of a columns-constant 64/64.
  - n-sum: one tiny bf16 matmul per M-tile against a block-indicator
    (bsel), accumulated in PSUM across the 16 M-tiles with a 2-tile lag.
  - divide by token count, DMA [64, 8] C-scores + [128, NBP] raw LSE sums.

Why this split: only VectorE (0.96 GHz) and ScalarE (1.2 GHz) can read PSUM
(1 elem/lane/cycle each); GPSIMD/DMA have no PSUM port on trn2.  The exact
max needs every element through VectorE (pair-TTR) AND half through ScalarE
(copy), saturating both at ~690 ns/doc.  The LSE trick lets each engine
reduce whole docs alone, decoupling them (~610 ns/doc combined).
"""

import numpy as np

B, NTOK, DIM = 64, 32, 128
C, S = 64, 1024
NCORES = 8
CSHARD = C // NCORES              # 8 docs per core
ROWS = B * NTOK                   # 2048 score rows
MTILES = ROWS // 128              # 16
DCOLS = CSHARD * S                # 8192 doc-token columns per core
TEMPERATURE = 0.02
BETA = 100.0                      # smooth-max sharpness (B path)
ABIAS = -90.0                     # exp argument bias: exp(BETA*x + ABIAS)

# per-M-tile doc order: interleave so both consumer engines stay busy
DOC_ORDER = (0, 4, 1, 5, 2, 6, 3, 7)
# B-set per M-tile: docs handled by the ScalarE LSE path.  60 B-pairs /
# 68 C-pairs total matches the measured engine pitches (1416 vs 1300 ns).
def _bset(m):
    return (4, 5, 6, 7)

BPAIRS = [(m, c) for m in range(MTILES) for c in _bset(m)]
BCOL = {mc: i for i, mc in enumerate(BPAIRS)}
NBP = len(BPAIRS)                 # 60

_CACHE = {}


def _build_nc():
    import concourse.bacc as bacc
    import concourse.tile as tile
    from concourse import mybir

    f32 = mybir.dt.float32
    bf16 = mybir.dt.bfloat16
    X = mybir.AxisListType.X
    MAX = mybir.AluOpType.max
    AF = mybir.ActivationFunctionType

    nc = bacc.Bacc("TRN2", target_bir_lowering=False, debug=False)
    qT_d = nc.dram_tensor("qT", [DIM, ROWS], bf16, kind="ExternalInput").ap()
    dT_d = nc.dram_tensor("dT", [DIM, DCOLS], bf16, kind="ExternalInput").ap()
    q0_d = nc.dram_tensor("q0t", [B, NTOK], f32, kind="ExternalInput").ap()
    bsel_d = nc.dram_tensor("bsel", [128, 124], bf16, kind="ExternalInput").ap()
    out_d = nc.dram_tensor("part", [B, CSHARD], f32, kind="ExternalOutput").ap()
    lse_d = nc.dram_tensor("lse", [128, NBP], f32, kind="ExternalOutput").ap()

    with tile.TileContext(nc) as tc:
        with (
            tc.tile_pool(name="const", bufs=1) as cpool,
            tc.tile_pool(name="trash", bufs=2) as trash_pool,
            tc.tile_pool(name="dmax", bufs=MTILES + 1) as dmax_pool,
            tc.tile_pool(name="lse", bufs=3) as lse_pool,
            tc.tile_pool(name="small", bufs=1) as small_pool,
        ):
            qT_sb = cpool.tile([DIM, ROWS], bf16)
            dT_sb = cpool.tile([DIM, DCOLS], bf16)
            bsel_sb = cpool.tile([128, 124], bf16)
            q0_sb = cpool.tile([B, NTOK], f32)

            wsb = cpool.tile([128, 512], bf16)
            nc.gpsimd.memset(wsb[:], 0.0)
            bias_t = cpool.tile([128, 1], f32)
            nc.gpsimd.memset(bias_t[:], ABIAS)
            nc.sync.dma_start(qT_sb[:, 0:128], qT_d[:, 0:128])
            # first doc in two half-chunks (consumers can start sooner), the
            # rest one chunk per doc in CONSUMPTION order so the PE's m=0
            # pass never stalls more than ~1 chunk behind the stream
            nc.sync.dma_start(dT_sb[:, 0:512], dT_d[:, 0:512])
            nc.sync.dma_start(dT_sb[:, 512:1024], dT_d[:, 512:1024])
            for j in DOC_ORDER[1:]:
                nc.sync.dma_start(
                    dT_sb[:, j * 1024:(j + 1) * 1024], dT_d[:, j * 1024:(j + 1) * 1024]
                )
            nc.gpsimd.dma_start(qT_sb[:, 128:2048], qT_d[:, 128:2048])
            nc.gpsimd.dma_start(q0_sb[:], q0_d[:])
            nc.gpsimd.dma_start(bsel_sb[:], bsel_d[:])

            # preload the Exp ACT table set (~2.7us) during the DMA ramp so
            # the first real exp doesn't pay it in-line
            warm_act = small_pool.tile([128, 1], f32)
            nc.scalar.activation(
                warm_act[:], bias_t[:], AF.Exp, bias=bias_t[:], scale=0.0
            )

            # lengths: count of query tokens with q[b, n, 0] != 0
            nz = small_pool.tile([B, NTOK], f32)
            nc.vector.tensor_scalar(
                nz[:], q0_sb[:], 0.0, None, op0=mybir.AluOpType.not_equal
            )
            lens = small_pool.tile([B, 1], f32)
            nc.vector.tensor_reduce(lens[:], nz[:], axis=X, op=mybir.AluOpType.add)
            rlen = small_pool.tile([B, 1], f32)
            nc.vector.reciprocal(rlen[:], lens[:])

            with (
                tc.tile_pool(name="psd", bufs=3, space="PSUM") as psd_pool,
                tc.tile_pool(name="aux", bufs=1, space="PSUM") as aux_pool,
            ):
                # HAM warm-up: a few dummy matmuls while the input DMAs run
                warm_ps = aux_pool.tile([128, 512], f32)
                for _ in range(16):
                    nc.tensor.matmul(
                        warm_ps[:, 0:256], wsb[:, 0:128], wsb[:, 0:256],
                        start=True, stop=True,
                    )

                scores_ps = aux_pool.tile([B, CSHARD], f32)
                pending = []  # n-sum lags 2 M-tiles
                for m in range(MTILES):
                    lhsT = qT_sb[:, m * 128:(m + 1) * 128]
                    order = (4, 0, 5, 1, 6, 2, 7, 3) if m == MTILES - 1 else DOC_ORDER
                    fillers = 0
                    bset = _bset(m)
                    base = BCOL[(m, bset[0])]
                    lse = lse_pool.tile([128, len(bset)], f32, tag="lse")
                    dmax = dmax_pool.tile([128, CSHARD], bf16, tag="dmax")
                    # zero so B columns add nothing in the n-sum matmul
                    nc.gpsimd.memset(dmax[:], 0.0)
                    for c in order:
                        ps = psd_pool.tile([128, 2, 512], f32, tag="psd")
                        for h in range(2):
                            col = c * 1024 + h * 512
                            nc.tensor.matmul(
                                ps[:, h, :],
                                lhsT,
                                dT_sb[:, col:col + 512],
                                start=True,
                                stop=True,
                            )
                        psf = ps.rearrange("p a b -> p (a b)")
                        if c not in bset:
                            nc.vector.tensor_reduce(
                                dmax[:, c:c + 1], psf[:], axis=X, op=MAX
                            )
                        else:
                            i = BCOL[(m, c)]
                            # out in-place into the PSUM tile: ScalarE's PSUM
                            # port is faster than SBUF, and only accum_out is
                            # consumed (the exp values are discarded)
                            nc.scalar.activation(
                                psf[:],
                                psf[:],
                                AF.Exp,
                                bias=bias_t[:],
                                scale=BETA,
                                accum_out=lse[:, i - base:i - base + 1],
                            )
                        for _ in range(fillers):
                            nc.tensor.matmul(
                                warm_ps[:, 0:256], wsb[:, 0:128],
                                wsb[:, 0:256], start=True, stop=True,
                            )
                    nc.sync.dma_start(
                        lse_d[:, base:base + len(bset)], lse[:]
                    )
                    pending.append((dmax, m))
                    if len(pending) > 2:
                        pdm, pm = pending.pop(0)
                        nc.tensor.matmul(
                            scores_ps[:],
                            bsel_sb[:, 60 - 4 * pm:124 - 4 * pm],
                            pdm[:],
                            start=(pm == 0),
                            stop=False,
                        )
                for pdm, pm in pending:
                    nc.tensor.matmul(
                        scores_ps[:],
                        bsel_sb[:, 60 - 4 * pm:124 - 4 * pm],
                        pdm[:],
                        start=(pm == 0),
                        stop=(pm == MTILES - 1),
                    )
                sc2 = small_pool.tile([B, CSHARD], f32)
                nc.vector.tensor_scalar_mul(sc2[:], scores_ps[:], rlen[:])
                nc.sync.dma_start(out_d[:], sc2[:])

    nc.compile()
    return nc


def _host_inputs(q, d):
    import ml_dtypes

    bf = ml_dtypes.bfloat16
    qT = np.ascontiguousarray(q.transpose(2, 0, 1).reshape(DIM, ROWS)).astype(bf)
    q0t = np.ascontiguousarray(q[:, :, 0])
    p = np.arange(128)
    bsel = np.zeros((128, 124), np.float32)
    bsel[p, 60 + p // 32] = 1.0
    bsel = bsel.astype(bf)
    in_maps = []
    for k in range(NCORES):
        dTk = np.ascontiguousarray(
            d[k * CSHARD:(k + 1) * CSHARD].transpose(2, 0, 1).reshape(DIM, DCOLS)
        ).astype(bf)
        in_maps.append({"qT": qT, "dT": dTk, "q0t": q0t, "bsel": bsel})
    return in_maps


def _loss_from_scores(S_mat, offset):
    logits = (S_mat.astype(np.float64)) / TEMPERATURE
    m = logits.max(axis=1, keepdims=True)
    logp = logits - m - np.log(np.exp(logits - m).sum(axis=1, keepdims=True))
    labels = np.arange(B) + offset
    return np.float32(-np.mean(logp[np.arange(B), labels]))


def kernel(**inputs):
    from concourse import bass_utils

    q = np.ascontiguousarray(np.asarray(inputs["query_embeddings"], dtype=np.float32))
    d = np.ascontiguousarray(np.asarray(inputs["doc_embeddings"], dtype=np.float32))
    offset = int(np.asarray(inputs["offset"]))
    assert q.shape == (B, NTOK, DIM) and d.shape == (C, S, DIM)

    if "nc" not in _CACHE:
        _CACHE["nc"] = _build_nc()
    nc = _CACHE["nc"]

    in_maps = _host_inputs(q, d)
    res = bass_utils.run_bass_kernel_spmd(nc, in_maps, core_ids=list(range(NCORES)))

    lengths = (q[:, :, 0] != 0).sum(axis=1).astype(np.float64)  # (64,)
    S_mat = np.zeros((B, C), np.float64)
    for k in range(NCORES):
        part = np.asarray(res.results[k]["part"], np.float64)    # (64, 8): C sums
        lse = np.asarray(res.results[k]["lse"], np.float64)      # (128, NBP)
        Sk = part.copy()
        # smooth-max contributions: lse[p, BCOL[(m,c)]] is the exp-sum for
        # query-token row r = m*128+p, doc k*CSHARD+c
        sm = (np.log(np.maximum(lse, 1e-300)) - ABIAS) / BETA    # (128, NBP)
        for i, (m, c) in enumerate(BPAIRS):
            r = m * 128 + np.arange(128)                          # global rows
            b = r // NTOK
            np.add.at(Sk[:, c], b, sm[:, i] / lengths[b])
        S_mat[:, k * CSHARD:(k + 1) * CSHARD] = Sk
    return _loss_from_scores(S_mat, offset)
